# revision 17
# baseline (speedup 1.0000x reference)
"""BertLayer (attention + adapter + FFN + LayerNorm) Trainium2 Bass kernel.

Sharding: 8 cores, pure SPMD (no collectives). Core c handles batch b=c//4
and query rows [q0, q0+512) with q0=(c%4)*512. Each core computes K/V for
its full batch locally (replicated within the 4-core batch group), then
attention / adapter / FFN / LayerNorm for its 512 rows.

All on-chip compute is done in the "transposed" orientation (feature dim
on partitions, token dim on the free axis) so that every matmul has its
contraction dim on partitions and no on-device transposes are needed.
The host pre-transposes the activations/weights when sharding, and
transposes the per-core [768, 512] output shards back while gathering.

Matmul operands are fp16 (1 cycle/row on the PE; plain-fp32 matmuls are
4x slower AND their LDWEIGHTS struct only carries one sync-wait, which the
Tile scheduler exceeds). Accumulation stays fp32 in PSUM. Softmax is
computed on transposed scores [kpos, q]: exp via ScalarE with the mask,
the 1/sqrt(dh) scale and a -2 overflow guard (cancels in normalization)
folded into the activation bias/scale; the denominator comes for free
from a ones-column appended to V in the context matmul. The V bias is
applied after normalization (probs @ (v+bv) = probs@v + bv).
"""

import numpy as np

import concourse.bass as bass
import concourse.mybir as mybir
import concourse.tile as tile
from concourse import bacc
from concourse.bass_utils import run_bass_kernel_spmd
from contextlib import ExitStack

F32 = mybir.dt.float32
F16 = mybir.dt.float16
AF = mybir.ActivationFunctionType

B, S, H = 2, 2048, 768
NH, DH = 12, 64
FF = 3072
AD = 64
EPS = 1e-12
P = 128
KO = H // P          # 6 partition-tiles of the hidden dim
Q = 512              # query rows per core
NCORES = 8
NCH = 4              # kpos chunks (512 each)
CH = S // NCH        # 512
JT = CH // P         # 4 kpos 128-tiles per chunk
FFT = FF // P        # 24
VH = 65              # per-head V columns incl. ones column


def r(ap):
    return ap


def _build_nc():
    nc = bacc.Bacc(
        "TRN2",
        target_bir_lowering=False,
        debug=False,
        num_devices=NCORES,
    )

    def din(name, shape, dt=F32):
        return nc.dram_tensor(name, list(shape), dt, kind="ExternalInput").ap()

    xt = din("xt", (H, S), F16)        # hidden[b].T
    xtq = din("xtq", (H, Q), F16)      # hidden[b, q0:q0+Q].T
    maskt = din("maskt", (P, S // P))
    wqt = din("wqt", (H, H), F16)
    wkt = din("wkt", (H, H), F16)
    wvt = din("wvt", (H, H), F16)
    afit = din("afit", (H, AD), F16)
    aset = din("aset", (AD, H), F16)
    wit = din("wit", (H, FF), F16)
    wot = din("wot", (FF, H), F16)
    bq6 = din("bq6", (P, KO))
    bk6 = din("bk6", (P, KO))
    bv6 = din("bv6", (P, KO))
    afib = din("afib", (AD, 1))
    aseb6 = din("aseb6", (P, KO))
    bi24 = din("bi24", (P, FFT))
    bo6 = din("bo6", (P, KO))
    lng6 = din("lng6", (P, KO))
    lnb6 = din("lnb6", (P, KO))
    outt = nc.dram_tensor("outt", [H, Q], F32, kind="ExternalOutput").ap()

    def part6(ap):  # [(ko p), n] -> [p, ko, n]
        return ap.rearrange("(ko p) n -> p ko n", p=P)

    with tile.TileContext(nc) as tc, nc.allow_low_precision(
        reason="float32r matmul operands; accumulation stays fp32 in PSUM"
    ), ExitStack() as top:
        const = top.enter_context(tc.tile_pool(name="const", bufs=1))
        persist = top.enter_context(tc.tile_pool(name="persist", bufs=1))

        # ---- constants ----
        maskt_sb = const.tile([P, S // P], F32, tag="maskt")
        nc.sync.dma_start(maskt_sb[:], maskt)
        bq_sb = const.tile([P, KO], F32, tag="bq")
        nc.sync.dma_start(bq_sb[:], bq6)
        bk_sb = const.tile([P, KO], F32, tag="bk")
        nc.sync.dma_start(bk_sb[:], bk6)
        aseb_sb = const.tile([P, KO], F32, tag="aseb")
        nc.sync.dma_start(aseb_sb[:], aseb6)
        bi_sb = const.tile([P, FFT], F32, tag="bi")
        nc.sync.dma_start(bi_sb[:], bi24)
        bo_sb = const.tile([P, KO], F32, tag="bo")
        nc.sync.dma_start(bo_sb[:], bo6)
        lng_sb = const.tile([P, KO], F32, tag="lng")
        nc.sync.dma_start(lng_sb[:], lng6)
        lnb_sb = const.tile([P, KO], F32, tag="lnb")
        nc.sync.dma_start(lnb_sb[:], lnb6)
        afib_sb = const.tile([AD, 1], F32, tag="afib")
        nc.sync.dma_start(afib_sb[:], afib)
        bv_sb = const.tile([P, KO], F32, tag="bv")
        nc.sync.dma_start(bv_sb[:], bv6)
        ones_col = const.tile([P, 1], F16, tag="ones")
        nc.vector.memset(ones_col[:], 1.0)
        ones_row = const.tile([1, P], F16, tag="ones_row")
        nc.vector.memset(ones_row[:], 1.0)

        # normalized attention output, transposed [H, Q]
        attn_outT = persist.tile([P, KO, Q], F16, tag="attn_outT")

        # ================= stage 0 + 1: QKV projections + attention ======
        with ExitStack() as s01:
            xt_pool = s01.enter_context(tc.tile_pool(name="xt", bufs=1))
            qt_pool = s01.enter_context(tc.tile_pool(name="qt", bufs=1))
            xt_sb = xt_pool.tile([P, KO, S], F16, tag="xt")
            nc.sync.dma_start(xt_sb[:], part6(xt))
            QT = qt_pool.tile([P, KO, Q], F16, tag="QT")

            # --- stage 0: QT = wq.T-contracted projection of query slice
            with ExitStack() as s0:
                w0_pool = s0.enter_context(tc.tile_pool(name="w0", bufs=1))
                p0_pool = s0.enter_context(
                    tc.tile_pool(name="p0", bufs=2, space="PSUM")
                )
                wq_sb = w0_pool.tile([P, KO, H], F16, tag="wq")
                nc.sync.dma_start(wq_sb[:], part6(wqt))
                xtq_sb = w0_pool.tile([P, KO, Q], F16, tag="xtq")
                nc.sync.dma_start(xtq_sb[:], part6(xtq))
                for ko in range(KO):
                    qp = p0_pool.tile([P, Q], F32, tag="qp")
                    for k in range(KO):
                        nc.tensor.matmul(
                            qp[:],
                            r(wq_sb[:, k, ko * P:(ko + 1) * P]),
                            r(xtq_sb[:, k, :]),
                            start=(k == 0),
                            stop=(k == KO - 1),
                        )
                    nc.scalar.activation(
                        QT[:, ko, :], qp[:], AF.Identity, bias=bq_sb[:, ko:ko + 1]
                    )

            # --- stage 1: K/V chunks + attention over kpos chunks
            wk_pool = s01.enter_context(tc.tile_pool(name="wk", bufs=1))
            wv_pool = s01.enter_context(tc.tile_pool(name="wv", bufs=1))
            wk_sb = wk_pool.tile([P, KO, H], F16, tag="wk")
            nc.sync.dma_start(wk_sb[:], part6(wkt))
            wv_sb = wv_pool.tile([P, KO, H], F16, tag="wv")
            nc.sync.dma_start(wv_sb[:], part6(wvt))

            acc_pool = s01.enter_context(tc.tile_pool(name="acc", bufs=1))
            ctx6 = acc_pool.tile([P, KO, Q], F32, tag="ctx6")
            den6 = acc_pool.tile([P, KO, Q], F32, tag="den6")
            nc.vector.memset(ctx6[:], 0.0)
            nc.vector.memset(den6[:], 0.0)

            kt_pool = s01.enter_context(tc.tile_pool(name="kt", bufs=1))
            vp_pool = s01.enter_context(tc.tile_pool(name="vp", bufs=1))
            et_pool = s01.enter_context(tc.tile_pool(name="et", bufs=2))
            mmp = s01.enter_context(tc.tile_pool(name="mmp", bufs=2, space="PSUM"))
            vpp = s01.enter_context(tc.tile_pool(name="vpp", bufs=2, space="PSUM"))
            cxp = s01.enter_context(tc.tile_pool(name="cxp", bufs=2, space="PSUM"))

            for c in range(NCH):
                # K^T chunk [H, CH]
                kt = kt_pool.tile([P, KO, CH], F16, tag="kt")
                for ko in range(KO):
                    kp = mmp.tile([P, CH], F32, tag="mm")
                    for k in range(KO):
                        nc.tensor.matmul(
                            kp[:],
                            r(wk_sb[:, k, ko * P:(ko + 1) * P]),
                            r(xt_sb[:, k, c * CH:(c + 1) * CH]),
                            start=(k == 0),
                            stop=(k == KO - 1),
                        )
                    nc.scalar.activation(
                        kt[:, ko, :], kp[:], AF.Identity, bias=bk_sb[:, ko:ko + 1]
                    )

                # V chunk, natural orientation, heads padded with ones col
                vp = vp_pool.tile([P, JT, NH, VH], F16, tag="vp")
                nc.vector.memset(vp[:, :, :, AD], 1.0)
                for j in range(JT):
                    s0_ = c * CH + j * P
                    for half in range(2):
                        vq = vpp.tile([P, 6 * AD], F32, tag="vq")
                        for k in range(KO):
                            nc.tensor.matmul(
                                vq[:],
                                r(xt_sb[:, k, s0_:s0_ + P]),
                                r(wv_sb[:, k, half * 6 * AD:(half + 1) * 6 * AD]),
                                start=(k == 0),
                                stop=(k == KO - 1),
                            )
                        nc.vector.tensor_copy(
                            vp[:, j, half * 6:(half + 1) * 6, 0:AD],
                            vq[:].rearrange("p (h d) -> p h d", d=AD),
                        )

                # attention for all heads against this kpos chunk
                for h in range(NH):
                    po = (h % 2) * DH
                    fo = h // 2
                    et = et_pool.tile([P, JT, Q], F16, tag="et")
                    for j in range(JT):
                        sp = mmp.tile([P, Q], F32, tag="mm")
                        nc.tensor.matmul(
                            sp[:],
                            r(kt[po:po + DH, fo, j * P:(j + 1) * P]),
                            r(QT[po:po + DH, fo, :]),
                            start=True,
                            stop=True,
                        )
                        nc.scalar.activation(
                            et[:, j, :],
                            sp[:],
                            AF.Exp,
                            bias=maskt_sb[:, c * JT + j:c * JT + j + 1],
                            scale=0.125,
                        )
                    cp = cxp.tile([VH, Q], F32, tag="cx")
                    for j in range(JT):
                        nc.tensor.matmul(
                            cp[:],
                            r(vp[:, j, h, :]),
                            r(et[:, j, :]),
                            start=(j == 0),
                            stop=(j == JT - 1),
                        )
                    nc.vector.tensor_add(
                        ctx6[po:po + DH, fo, :], ctx6[po:po + DH, fo, :], cp[0:AD, :]
                    )
                    nc.vector.tensor_add(
                        den6[po:po + 1, fo, :], den6[po:po + 1, fo, :], cp[AD:VH, :]
                    )

            # normalize: attn_outT = ctx / den  (per head); the reciprocal
            # row is broadcast across partitions with a K=1 ones matmul.
            nrm_pool = s01.enter_context(tc.tile_pool(name="nrm", bufs=2))
            bbp = s01.enter_context(tc.tile_pool(name="bbp", bufs=2, space="PSUM"))
            for h in range(NH):
                po = (h % 2) * DH
                fo = h // 2
                rc = nrm_pool.tile([1, Q], F16, tag="rc")
                nc.vector.reciprocal(rc[:], den6[po:po + 1, fo, :])
                bc = bbp.tile([P, Q], F32, tag="bc")
                nc.tensor.matmul(bc[:], r(ones_row[:]), r(rc[:]), start=True, stop=True)
                nc.vector.tensor_mul(
                    attn_outT[po:po + DH, fo, :], ctx6[po:po + DH, fo, :], bc[0:DH, :]
                )
                nc.vector.tensor_scalar_add(
                    attn_outT[po:po + DH, fo, :],
                    attn_outT[po:po + DH, fo, :],
                    bv_sb[po:po + DH, fo:fo + 1],
                )

        # ================= stage 2 + 3: adapter + FFN + LayerNorm =========
        with ExitStack() as s23:
            small = s23.enter_context(tc.tile_pool(name="small", bufs=1))
            big23 = s23.enter_context(tc.tile_pool(name="big23", bufs=1))
            aw_pool = s23.enter_context(tc.tile_pool(name="aw", bufs=1))
            mid_pool = s23.enter_context(tc.tile_pool(name="mid", bufs=2))
            ps = s23.enter_context(tc.tile_pool(name="ps", bufs=2, space="PSUM"))

            afit_sb = aw_pool.tile([P, KO, AD], F16, tag="afit")
            nc.sync.dma_start(afit_sb[:], part6(afit))
            aset_sb = aw_pool.tile([AD, H], F16, tag="aset")
            nc.sync.dma_start(aset_sb[:], aset)

            # adapter down-projection + gelu -> aT [AD, Q]
            ap_ps = ps.tile([AD, Q], F32, tag="ps512")
            for k in range(KO):
                nc.tensor.matmul(
                    ap_ps[:],
                    r(afit_sb[:, k, :]),
                    r(attn_outT[:, k, :]),
                    start=(k == 0),
                    stop=(k == KO - 1),
                )
            aT = mid_pool.tile([AD, Q], F16, tag="aT")
            nc.scalar.activation(aT[:], ap_ps[:], AF.Gelu, bias=afib_sb[:])

            # adapter up-projection + residual -> attn2T [H, Q]
            attn2T = big23.tile([P, KO, Q], F16, tag="attn2T")
            for m in range(KO):
                pp = ps.tile([P, Q], F32, tag="ps512")
                nc.tensor.matmul(
                    pp[:],
                    r(aset_sb[:, m * P:(m + 1) * P]),
                    r(aT[:]),
                    start=True,
                    stop=True,
                )
                nc.vector.scalar_tensor_tensor(
                    attn2T[:, m, :],
                    pp[:],
                    aseb_sb[:, m:m + 1],
                    attn_outT[:, m, :],
                    mybir.AluOpType.add,
                    mybir.AluOpType.add,
                )

            # FFN
            yT = big23.tile([P, KO, Q], F16, tag="yT")
            with ExitStack() as ffn:
                ypool = ffn.enter_context(
                    tc.tile_pool(name="yp", bufs=1, space="PSUM")
                )
                wi_pool = ffn.enter_context(tc.tile_pool(name="wi", bufs=2))
                wo_pool = ffn.enter_context(tc.tile_pool(name="wo", bufs=3))
                it_pool = ffn.enter_context(tc.tile_pool(name="it", bufs=3))

                ytiles = [
                    ypool.tile([P, Q], F32, tag=f"y{m}", name=f"y{m}")
                    for m in range(KO)
                ]
                for kc in range(KO):  # 6 chunks of 512 FF rows
                    wchunk = wi_pool.tile([P, KO, 512], F16, tag="wchunk")
                    nc.sync.dma_start(
                        wchunk[:], part6(wit[:, kc * 512:(kc + 1) * 512])
                    )
                    for kk in range(4):
                        k = kc * 4 + kk
                        ip = ps.tile([P, Q], F32, tag="ps512")
                        for k6 in range(KO):
                            nc.tensor.matmul(
                                ip[:],
                                r(wchunk[:, k6, kk * P:(kk + 1) * P]),
                                r(attn2T[:, k6, :]),
                                start=(k6 == 0),
                                stop=(k6 == KO - 1),
                            )
                        it = it_pool.tile([P, Q], F16, tag="it")
                        nc.scalar.activation(
                            it[:], ip[:], AF.Gelu, bias=bi_sb[:, k:k + 1]
                        )
                        wo_t = wo_pool.tile([P, H], F16, tag="wo_t")
                        nc.sync.dma_start(wo_t[:], wot[k * P:(k + 1) * P, :])
                        for m in range(KO):
                            nc.tensor.matmul(
                                ytiles[m][:],
                                r(wo_t[:, m * P:(m + 1) * P]),
                                r(it[:]),
                                start=(k == 0),
                                stop=(k == FFT - 1),
                            )
                for m in range(KO):
                    nc.vector.scalar_tensor_tensor(
                        yT[:, m, :],
                        ytiles[m][:],
                        bo_sb[:, m:m + 1],
                        attn2T[:, m, :],
                        mybir.AluOpType.add,
                        mybir.AluOpType.add,
                    )

            # LayerNorm over H (partition dim across 6 tiles):
            # mean / mean-of-squares via ones-column matmuls
            lnp = s23.enter_context(tc.tile_pool(name="lnp", bufs=1, space="PSUM"))
            mu_ps = lnp.tile([1, Q], F32, tag="mu")
            for m in range(KO):
                nc.tensor.matmul(
                    mu_ps[:], r(ones_col[:]), r(yT[:, m, :]),
                    start=(m == 0), stop=(m == KO - 1),
                )
            sq_ps = lnp.tile([1, Q], F32, tag="sq")
            for m in range(KO):
                sqt = mid_pool.tile([P, Q], F16, tag="sqt")
                nc.scalar.activation(sqt[:], yT[:, m, :], AF.Square)
                nc.tensor.matmul(
                    sq_ps[:], r(ones_col[:]), r(sqt[:]),
                    start=(m == 0), stop=(m == KO - 1),
                )

            mu = small.tile([1, Q], F32, tag="mu_sb")
            nc.vector.tensor_scalar_mul(mu[:], mu_ps[:], 1.0 / H)
            ms = small.tile([1, Q], F32, tag="ms_sb")
            nc.vector.tensor_scalar_mul(ms[:], sq_ps[:], 1.0 / H)
            mu2 = small.tile([1, Q], F32, tag="mu2")
            nc.vector.tensor_mul(mu2[:], mu[:], mu[:])
            nc.vector.tensor_sub(ms[:], ms[:], mu2[:])  # variance
            eps_sb = small.tile([1, 1], F32, tag="eps")
            nc.vector.memset(eps_sb[:], EPS)
            sd = small.tile([1, Q], F32, tag="sd")
            nc.scalar.activation(sd[:], ms[:], AF.Sqrt, bias=eps_sb[:])
            rs = small.tile([1, Q], F16, tag="rs")
            nc.vector.reciprocal(rs[:], sd[:])
            nm = small.tile([1, Q], F16, tag="nm")
            nc.vector.tensor_mul(nm[:], mu[:], rs[:])
            nc.vector.tensor_scalar_mul(nm[:], nm[:], -1.0)
            bb23 = s23.enter_context(tc.tile_pool(name="bb23", bufs=1, space="PSUM"))
            Ab = bb23.tile([P, Q], F32, tag="Ab")
            nc.tensor.matmul(Ab[:], r(ones_row[:]), r(rs[:]), start=True, stop=True)
            Mb = bb23.tile([P, Q], F32, tag="Mb")
            nc.tensor.matmul(Mb[:], r(ones_row[:]), r(nm[:]), start=True, stop=True)

            outt_p = part6(outt)
            for m in range(KO):
                t1 = mid_pool.tile([P, Q], F32, tag="sqt")
                nc.vector.tensor_mul(t1[:], yT[:, m, :], Ab[:])
                nc.vector.tensor_add(t1[:], t1[:], Mb[:])
                ot = mid_pool.tile([P, Q], F32, tag="ot")
                nc.vector.scalar_tensor_tensor(
                    ot[:],
                    t1[:],
                    lng_sb[:, m:m + 1],
                    lnb_sb[:, m:m + 1].to_broadcast([P, Q]),
                    mybir.AluOpType.mult,
                    mybir.AluOpType.add,
                )
                nc.sync.dma_start(outt_p[:, m, :], ot[:])

    nc.compile()
    return nc


_NC_CACHE = None


def _get_nc():
    global _NC_CACHE
    if _NC_CACHE is None:
        _NC_CACHE = _build_nc()
    return _NC_CACHE


def make_in_maps(
    hidden_states, attention_mask, wq, bq, wk, bk, wv, bv,
    a_fi_w, a_fi_b, a_se_w, a_se_b, wi, bi, wo, bo, ln_g, ln_b,
):
    f = np.float32
    h16 = np.float16
    ca = np.ascontiguousarray

    def part_bias(v, n):  # [n*128] -> [128, n]
        return ca(np.asarray(v, f).reshape(n, P).T)

    shared = {
        "wqt": ca(np.asarray(wq, h16).T),
        "wkt": ca(np.asarray(wk, h16).T),
        "wvt": ca(np.asarray(wv, h16).T),
        "afit": ca(np.asarray(a_fi_w, h16).T),
        "aset": ca(np.asarray(a_se_w, h16).T),
        "wit": ca(np.asarray(wi, h16).T),
        "wot": ca(np.asarray(wo, h16).T),
        "bq6": part_bias(bq, KO),
        "bk6": part_bias(bk, KO),
        "bv6": part_bias(bv, KO),
        "afib": ca(np.asarray(a_fi_b, f).reshape(AD, 1)),
        "aseb6": part_bias(a_se_b, KO),
        "bi24": part_bias(bi, FFT),
        "bo6": part_bias(bo, KO),
        "lng6": part_bias(ln_g, KO),
        "lnb6": part_bias(ln_b, KO),
    }
    hs = np.asarray(hidden_states)
    am = np.asarray(attention_mask, f)
    in_maps = []
    for c in range(NCORES):
        b = c // (NCORES // B)
        q0 = (c % (NCORES // B)) * Q
        m = dict(shared)
        m["xt"] = ca(hs[b].T.astype(h16))
        m["xtq"] = ca(hs[b, q0:q0 + Q].T.astype(h16))
        # -2.0: guard against fp16 overflow of exp(); cancels in softmax
        m["maskt"] = ca((am[b, 0, 0].reshape(S // P, P).T - 2.0))
        in_maps.append(m)
    return in_maps


def gather_out(results):
    out = np.empty((B, S, H), np.float32)
    for c in range(NCORES):
        b = c // (NCORES // B)
        q0 = (c % (NCORES // B)) * Q
        out[b, q0:q0 + Q, :] = results[c]["outt"].T
    return out


def kernel(**inputs):
    nc = _get_nc()
    in_maps = make_in_maps(**inputs)
    res = run_bass_kernel_spmd(nc, in_maps, core_ids=list(range(NCORES)))
    return gather_out(res.results)


# revision 25
# speedup vs baseline: 1.0716x; 1.0716x over previous
"""BertLayer (attention + adapter + FFN + LayerNorm) Trainium2 Bass kernel.

Sharding: 8 cores, pure SPMD (no collectives). Core c handles batch b=c//4
and query rows [q0, q0+512) with q0=(c%4)*512. Each core computes K/V for
its full batch locally (replicated within the 4-core batch group), then
attention / adapter / FFN / LayerNorm for its 512 rows.

All on-chip compute is done in the "transposed" orientation (feature dim
on partitions, token dim on the free axis) so that every matmul has its
contraction dim on partitions and no on-device transposes are needed.
The host pre-transposes the activations/weights when sharding, and
transposes the per-core [768, 512] output shards back while gathering.

Matmul operands are fp16 (1 cycle/row on the PE; plain-fp32 matmuls are
4x slower AND their LDWEIGHTS struct only carries one sync-wait, which the
Tile scheduler exceeds). Accumulation stays fp32 in PSUM. Softmax is
computed on transposed scores [kpos, q]: exp via ScalarE with the mask,
the 1/sqrt(dh) scale and a -2 overflow guard (cancels in normalization)
folded into the activation bias/scale; the denominator comes for free
from a ones-column appended to V in the context matmul. The V bias is
applied after normalization (probs @ (v+bv) = probs@v + bv).
"""

import numpy as np

import concourse.bass as bass
import concourse.mybir as mybir
import concourse.tile as tile
from concourse import bacc
from concourse.bass_utils import run_bass_kernel_spmd
from contextlib import ExitStack

F32 = mybir.dt.float32
F16 = mybir.dt.float16
AF = mybir.ActivationFunctionType

B, S, H = 2, 2048, 768
NH, DH = 12, 64
FF = 3072
AD = 64
EPS = 1e-12
P = 128
KO = H // P          # 6 partition-tiles of the hidden dim
Q = 512              # query rows per core
NCORES = 8
NCH = 4              # kpos chunks (512 each)
CH = S // NCH        # 512
JT = CH // P         # 4 kpos 128-tiles per chunk
FFT = FF // P        # 24
VH = 65              # per-head V columns incl. ones column


def r(ap):
    return ap


def _build_nc():
    nc = bacc.Bacc(
        "TRN2",
        target_bir_lowering=False,
        debug=False,
        num_devices=NCORES,
    )

    def din(name, shape, dt=F32):
        return nc.dram_tensor(name, list(shape), dt, kind="ExternalInput").ap()

    xt = din("xt", (H, S), F16)        # hidden[b].T
    xtq = din("xtq", (H, Q), F16)      # hidden[b, q0:q0+Q].T
    wqt = din("wqt", (H, H), F16)
    wkt = din("wkt", (H, H), F16)
    wvt = din("wvt", (H, H), F16)
    afit = din("afit", (H, AD), F16)
    aset = din("aset", (AD, H), F16)
    wit = din("wit", (H, FF), F16)
    wot = din("wot", (FF, H), F16)
    consts = din("consts", (P, 83))
    outt = nc.dram_tensor("outt", [H, Q], F32, kind="ExternalOutput").ap()

    def part6(ap):  # [(ko p), n] -> [p, ko, n]
        return ap.rearrange("(ko p) n -> p ko n", p=P)

    with tile.TileContext(nc) as tc, nc.allow_low_precision(
        reason="float32r matmul operands; accumulation stays fp32 in PSUM"
    ), ExitStack() as top:
        const = top.enter_context(tc.tile_pool(name="const", bufs=1))
        persist = top.enter_context(tc.tile_pool(name="persist", bufs=1))

        # ---- constants (single packed DMA; traced after the hot-path
        # weight loads so it does not delay the first matmuls) ----
        consts_sb = const.tile([P, 83], F32, tag="consts")
        maskt_sb = consts_sb[:, 0:16]
        bq_sb = consts_sb[:, 16:22]
        bk_sb = consts_sb[:, 22:28]
        bv_sb = consts_sb[:, 28:34]
        aseb_sb = consts_sb[:, 34:40]
        bo_sb = consts_sb[:, 40:46]
        lng_sb = consts_sb[:, 46:52]
        lnb_sb = consts_sb[:, 52:58]
        bi_sb = consts_sb[:, 58:82]
        afib_sb = consts_sb[0:AD, 82:83]
        ones_col = const.tile([P, 1], F16, tag="ones")
        nc.vector.memset(ones_col[:], 1.0)
        ones_row = const.tile([1, P], F16, tag="ones_row")
        nc.vector.memset(ones_row[:], 1.0)

        # normalized attention output, transposed [H, Q]
        attn_outT = persist.tile([P, KO, Q], F16, tag="attn_outT")

        # ================= stage 0 + 1: QKV projections + attention ======
        with ExitStack() as s01:
            xt_pool = s01.enter_context(tc.tile_pool(name="xt", bufs=1))
            qt_pool = s01.enter_context(tc.tile_pool(name="qt", bufs=1))
            QT = qt_pool.tile([P, KO, Q], F16, tag="QT")

            # --- stage 0: QT = wq.T-contracted projection of query slice
            with ExitStack() as s0:
                w0_pool = s0.enter_context(tc.tile_pool(name="w0", bufs=1))
                p0_pool = s0.enter_context(
                    tc.tile_pool(name="p0", bufs=2, space="PSUM")
                )
                wqc, xtqc = [], []
                for k in range(KO):
                    wt = w0_pool.tile([P, H], F16, tag=f"wq{k}", name=f"wq{k}")
                    nc.sync.dma_start(wt[:], wqt[k * P:(k + 1) * P, :])
                    wqc.append(wt)
                    xq = w0_pool.tile([P, Q], F16, tag=f"xtq{k}", name=f"xtq{k}")
                    nc.sync.dma_start(xq[:], xtq[k * P:(k + 1) * P, :])
                    xtqc.append(xq)
                nc.sync.dma_start(consts_sb[:], consts)
                for ko in range(KO):
                    qp = p0_pool.tile([P, Q], F32, tag="qp")
                    for k in range(KO):
                        nc.tensor.matmul(
                            qp[:],
                            r(wqc[k][:, ko * P:(ko + 1) * P]),
                            r(xtqc[k][:]),
                            start=(k == 0),
                            stop=(k == KO - 1),
                        )
                    nc.scalar.activation(
                        QT[:, ko, :], qp[:], AF.Identity, bias=bq_sb[:, ko:ko + 1]
                    )

            # --- stage 1: K/V chunks + attention over kpos chunks
            wk_pool = s01.enter_context(tc.tile_pool(name="wk", bufs=1))
            wv_pool = s01.enter_context(tc.tile_pool(name="wv", bufs=1))
            wkc, wvc = [], []
            for k in range(KO):
                wt = wk_pool.tile([P, H], F16, tag=f"wk{k}", name=f"wk{k}")
                nc.sync.dma_start(wt[:], wkt[k * P:(k + 1) * P, :])
                wkc.append(wt)
                wt = wv_pool.tile([P, H], F16, tag=f"wv{k}", name=f"wv{k}")
                nc.sync.dma_start(wt[:], wvt[k * P:(k + 1) * P, :])
                wvc.append(wt)
            xtp = part6(xt)
            xtc = []
            for c in range(NCH):
                t = xt_pool.tile([P, KO, CH], F16, tag=f"xt{c}", name=f"xt{c}")
                nc.sync.dma_start(t[:], xtp[:, :, c * CH:(c + 1) * CH])
                xtc.append(t)

            kt_pool = s01.enter_context(tc.tile_pool(name="kt", bufs=1))
            vp_pool = s01.enter_context(tc.tile_pool(name="vp", bufs=1))
            et_pool = s01.enter_context(tc.tile_pool(name="et", bufs=4))
            mmp = s01.enter_context(tc.tile_pool(name="mmp", bufs=3, space="PSUM"))
            vpp = s01.enter_context(tc.tile_pool(name="vpp", bufs=2, space="PSUM"))
            cxp = s01.enter_context(tc.tile_pool(name="cxp", bufs=2, space="PSUM"))
            bcp = s01.enter_context(tc.tile_pool(name="bcp", bufs=1, space="PSUM"))

            # phase A: K^T and V for the full sequence, kept resident in
            # per-chunk tiles (chunk granularity lets attention on chunk c
            # start while chunk c+1 is still being projected)
            kts, vps = [], []
            for c in range(NCH):
                kt = kt_pool.tile([P, KO, CH], F16, tag=f"kt{c}", name=f"kt{c}")
                for ko in range(KO):
                    kp = mmp.tile([P, CH], F32, tag="mm")
                    for k in range(KO):
                        nc.tensor.matmul(
                            kp[:],
                            r(wkc[k][:, ko * P:(ko + 1) * P]),
                            r(xtc[c][:, k, :]),
                            start=(k == 0),
                            stop=(k == KO - 1),
                        )
                    nc.scalar.activation(
                        kt[:, ko, :], kp[:], AF.Identity, bias=bk_sb[:, ko:ko + 1]
                    )
                vp = vp_pool.tile([P, JT, NH, VH], F16, tag=f"vp{c}", name=f"vp{c}")
                nc.vector.memset(vp[:, :, :, AD], 1.0)
                for j in range(JT):
                    for half in range(2):
                        vq = vpp.tile([P, 6 * AD], F32, tag="vq")
                        for k in range(KO):
                            nc.tensor.matmul(
                                vq[:],
                                r(xtc[c][:, k, j * P:(j + 1) * P]),
                                r(wvc[k][:, half * 6 * AD:(half + 1) * 6 * AD]),
                                start=(k == 0),
                                stop=(k == KO - 1),
                            )
                        nc.vector.tensor_copy(
                            vp[:, j, half * 6:(half + 1) * 6, 0:AD],
                            vq[:].rearrange("p (h d) -> p h d", d=AD),
                        )
                kts.append(kt)
                vps.append(vp)

            # phase B: per head, accumulate context (and the softmax
            # denominator via the ones column of V) across all 16 kpos
            # tiles in a single PSUM bank, then normalize on eviction
            nrm_pool = s01.enter_context(tc.tile_pool(name="nrm", bufs=2))
            for h in range(NH):
                po = (h % 2) * DH
                fo = h // 2
                cp = cxp.tile([VH, Q], F32, tag="cx")
                for c in range(NCH):
                    for j in range(JT):
                        jj = c * JT + j
                        sp = mmp.tile([P, Q], F32, tag="mm")
                        nc.tensor.matmul(
                            sp[:],
                            r(kts[c][po:po + DH, fo, j * P:(j + 1) * P]),
                            r(QT[po:po + DH, fo, :]),
                            start=True,
                            stop=True,
                        )
                        et = et_pool.tile([P, Q], F16, tag="et")
                        nc.scalar.activation(
                            et[:],
                            sp[:],
                            AF.Exp,
                            bias=maskt_sb[:, jj:jj + 1],
                            scale=0.125,
                        )
                        nc.tensor.matmul(
                            cp[:],
                            r(vps[c][:, j, h, :]),
                            r(et[:]),
                            start=(jj == 0),
                            stop=(jj == NCH * JT - 1),
                        )
                rc = nrm_pool.tile([1, Q], F16, tag="rc")
                nc.vector.reciprocal(rc[:], cp[AD:VH, :])
                bc = bcp.tile([P, Q], F32, tag="bc")
                nc.tensor.matmul(bc[:], r(ones_row[:]), r(rc[:]), start=True, stop=True)
                bcs = nrm_pool.tile([DH, Q], F32, tag="bcs")
                nc.scalar.activation(bcs[:], bc[0:DH, :], AF.Copy)
                nc.vector.tensor_mul(
                    attn_outT[po:po + DH, fo, :], cp[0:AD, :], bcs[:]
                )
                nc.vector.tensor_scalar_add(
                    attn_outT[po:po + DH, fo, :],
                    attn_outT[po:po + DH, fo, :],
                    bv_sb[po:po + DH, fo:fo + 1],
                )

        # ================= stage 2 + 3: adapter + FFN + LayerNorm =========
        with ExitStack() as s23:
            small = s23.enter_context(tc.tile_pool(name="small", bufs=1))
            big23 = s23.enter_context(tc.tile_pool(name="big23", bufs=1))
            aw_pool = s23.enter_context(tc.tile_pool(name="aw", bufs=1))
            mid_pool = s23.enter_context(tc.tile_pool(name="mid", bufs=2))
            ps = s23.enter_context(tc.tile_pool(name="ps", bufs=2, space="PSUM"))

            afit_sb = aw_pool.tile([P, KO, AD], F16, tag="afit")
            nc.sync.dma_start(afit_sb[:], part6(afit))
            aset_sb = aw_pool.tile([AD, H], F16, tag="aset")
            nc.sync.dma_start(aset_sb[:], aset)

            # adapter down-projection + gelu -> aT [AD, Q]
            ap_ps = ps.tile([AD, Q], F32, tag="ps512")
            for k in range(KO):
                nc.tensor.matmul(
                    ap_ps[:],
                    r(afit_sb[:, k, :]),
                    r(attn_outT[:, k, :]),
                    start=(k == 0),
                    stop=(k == KO - 1),
                )
            aT = mid_pool.tile([AD, Q], F16, tag="aT")
            nc.scalar.activation(aT[:], ap_ps[:], AF.Gelu, bias=afib_sb[:])

            # adapter up-projection + residual -> attn2T [H, Q]
            attn2T = big23.tile([P, KO, Q], F16, tag="attn2T")
            for m in range(KO):
                pp = ps.tile([P, Q], F32, tag="ps512")
                nc.tensor.matmul(
                    pp[:],
                    r(aset_sb[:, m * P:(m + 1) * P]),
                    r(aT[:]),
                    start=True,
                    stop=True,
                )
                nc.vector.scalar_tensor_tensor(
                    attn2T[:, m, :],
                    pp[:],
                    aseb_sb[:, m:m + 1],
                    attn_outT[:, m, :],
                    mybir.AluOpType.add,
                    mybir.AluOpType.add,
                )

            # FFN
            yT = big23.tile([P, KO, Q], F16, tag="yT")
            with ExitStack() as ffn:
                ypool = ffn.enter_context(
                    tc.tile_pool(name="yp", bufs=1, space="PSUM")
                )
                wi_pool = ffn.enter_context(tc.tile_pool(name="wi", bufs=2))
                wo_pool = ffn.enter_context(tc.tile_pool(name="wo", bufs=3))
                it_pool = ffn.enter_context(tc.tile_pool(name="it", bufs=3))

                ytiles = [
                    ypool.tile([P, Q], F32, tag=f"y{m}", name=f"y{m}")
                    for m in range(KO)
                ]
                for kc in range(KO):  # 6 chunks of 512 FF rows
                    wchunk = wi_pool.tile([P, KO, 512], F16, tag="wchunk")
                    nc.sync.dma_start(
                        wchunk[:], part6(wit[:, kc * 512:(kc + 1) * 512])
                    )
                    for kk in range(4):
                        k = kc * 4 + kk
                        ip = ps.tile([P, Q], F32, tag="ps512")
                        for k6 in range(KO):
                            nc.tensor.matmul(
                                ip[:],
                                r(wchunk[:, k6, kk * P:(kk + 1) * P]),
                                r(attn2T[:, k6, :]),
                                start=(k6 == 0),
                                stop=(k6 == KO - 1),
                            )
                        it = it_pool.tile([P, Q], F16, tag="it")
                        nc.scalar.activation(
                            it[:], ip[:], AF.Gelu, bias=bi_sb[:, k:k + 1]
                        )
                        wo_t = wo_pool.tile([P, H], F16, tag="wo_t")
                        nc.sync.dma_start(wo_t[:], wot[k * P:(k + 1) * P, :])
                        for m in range(KO):
                            nc.tensor.matmul(
                                ytiles[m][:],
                                r(wo_t[:, m * P:(m + 1) * P]),
                                r(it[:]),
                                start=(k == 0),
                                stop=(k == FFT - 1),
                            )
                for m in range(KO):
                    nc.vector.scalar_tensor_tensor(
                        yT[:, m, :],
                        ytiles[m][:],
                        bo_sb[:, m:m + 1],
                        attn2T[:, m, :],
                        mybir.AluOpType.add,
                        mybir.AluOpType.add,
                    )

            # LayerNorm over H (partition dim across 6 tiles):
            # mean / mean-of-squares via ones-column matmuls
            lnp = s23.enter_context(tc.tile_pool(name="lnp", bufs=1, space="PSUM"))
            mu_ps = lnp.tile([1, Q], F32, tag="mu")
            for m in range(KO):
                nc.tensor.matmul(
                    mu_ps[:], r(ones_col[:]), r(yT[:, m, :]),
                    start=(m == 0), stop=(m == KO - 1),
                )
            sq_ps = lnp.tile([1, Q], F32, tag="sq")
            for m in range(KO):
                sqt = mid_pool.tile([P, Q], F16, tag="sqt")
                nc.scalar.activation(sqt[:], yT[:, m, :], AF.Square)
                nc.tensor.matmul(
                    sq_ps[:], r(ones_col[:]), r(sqt[:]),
                    start=(m == 0), stop=(m == KO - 1),
                )

            mu = small.tile([1, Q], F32, tag="mu_sb")
            nc.vector.tensor_scalar_mul(mu[:], mu_ps[:], 1.0 / H)
            ms = small.tile([1, Q], F32, tag="ms_sb")
            nc.vector.tensor_scalar_mul(ms[:], sq_ps[:], 1.0 / H)
            mu2 = small.tile([1, Q], F32, tag="mu2")
            nc.vector.tensor_mul(mu2[:], mu[:], mu[:])
            nc.vector.tensor_sub(ms[:], ms[:], mu2[:])  # variance
            eps_sb = small.tile([1, 1], F32, tag="eps")
            nc.vector.memset(eps_sb[:], EPS)
            sd = small.tile([1, Q], F32, tag="sd")
            nc.scalar.activation(sd[:], ms[:], AF.Sqrt, bias=eps_sb[:])
            rs = small.tile([1, Q], F16, tag="rs")
            nc.vector.reciprocal(rs[:], sd[:])
            nm = small.tile([1, Q], F16, tag="nm")
            nc.vector.tensor_mul(nm[:], mu[:], rs[:])
            nc.vector.tensor_scalar_mul(nm[:], nm[:], -1.0)
            bb23 = s23.enter_context(tc.tile_pool(name="bb23", bufs=1, space="PSUM"))
            Ab = bb23.tile([P, Q], F32, tag="Ab")
            nc.tensor.matmul(Ab[:], r(ones_row[:]), r(rs[:]), start=True, stop=True)
            Mb = bb23.tile([P, Q], F32, tag="Mb")
            nc.tensor.matmul(Mb[:], r(ones_row[:]), r(nm[:]), start=True, stop=True)

            outt_p = part6(outt)
            for m in range(KO):
                t1 = mid_pool.tile([P, Q], F32, tag="sqt")
                nc.vector.tensor_mul(t1[:], yT[:, m, :], Ab[:])
                nc.vector.tensor_add(t1[:], t1[:], Mb[:])
                ot = mid_pool.tile([P, Q], F32, tag="ot")
                nc.vector.scalar_tensor_tensor(
                    ot[:],
                    t1[:],
                    lng_sb[:, m:m + 1],
                    lnb_sb[:, m:m + 1].to_broadcast([P, Q]),
                    mybir.AluOpType.mult,
                    mybir.AluOpType.add,
                )
                nc.sync.dma_start(outt_p[:, m, :], ot[:])

    nc.compile()
    return nc


_NC_CACHE = None


def _get_nc():
    global _NC_CACHE
    if _NC_CACHE is None:
        _NC_CACHE = _build_nc()
    return _NC_CACHE


def make_in_maps(
    hidden_states, attention_mask, wq, bq, wk, bk, wv, bv,
    a_fi_w, a_fi_b, a_se_w, a_se_b, wi, bi, wo, bo, ln_g, ln_b,
):
    f = np.float32
    h16 = np.float16
    ca = np.ascontiguousarray

    def part_bias(v, n):  # [n*128] -> [128, n]
        return ca(np.asarray(v, f).reshape(n, P).T)

    shared = {
        "wqt": ca(np.asarray(wq, h16).T),
        "wkt": ca(np.asarray(wk, h16).T),
        "wvt": ca(np.asarray(wv, h16).T),
        "afit": ca(np.asarray(a_fi_w, h16).T),
        "aset": ca(np.asarray(a_se_w, h16).T),
        "wit": ca(np.asarray(wi, h16).T),
        "wot": ca(np.asarray(wo, h16).T),
    }

    def _consts(mask_b):
        c = np.zeros((P, 83), f)
        c[:, 0:16] = mask_b.reshape(S // P, P).T - 2.0
        c[:, 16:22] = part_bias(bq, KO)
        c[:, 22:28] = part_bias(bk, KO)
        c[:, 28:34] = part_bias(bv, KO)
        c[:, 34:40] = part_bias(a_se_b, KO)
        c[:, 40:46] = part_bias(bo, KO)
        c[:, 46:52] = part_bias(ln_g, KO)
        c[:, 52:58] = part_bias(ln_b, KO)
        c[:, 58:82] = part_bias(bi, FFT)
        c[0:AD, 82] = np.asarray(a_fi_b, f)
        return c
    hs = np.asarray(hidden_states)
    am = np.asarray(attention_mask, f)
    in_maps = []
    for c in range(NCORES):
        b = c // (NCORES // B)
        q0 = (c % (NCORES // B)) * Q
        m = dict(shared)
        m["xt"] = ca(hs[b].T.astype(h16))
        m["xtq"] = ca(hs[b, q0:q0 + Q].T.astype(h16))
        # -2.0 in the mask guards against fp16 overflow of exp();
        # it cancels in the softmax normalization
        m["consts"] = _consts(am[b, 0, 0])
        in_maps.append(m)
    return in_maps


def gather_out(results):
    out = np.empty((B, S, H), np.float32)
    for c in range(NCORES):
        b = c // (NCORES // B)
        q0 = (c % (NCORES // B)) * Q
        out[b, q0:q0 + Q, :] = results[c]["outt"].T
    return out


def kernel(**inputs):
    nc = _get_nc()
    in_maps = make_in_maps(**inputs)
    res = run_bass_kernel_spmd(nc, in_maps, core_ids=list(range(NCORES)))
    return gather_out(res.results)
